# revision 18
# baseline (speedup 1.0000x reference)
"""Trainium2 Bass kernel for the masked Fisher-Kolmogorov reaction-diffusion
simulation (nn_DiscreteSimulation_65609920413847).

Math per step (per sample, while t_s > step):
    u' = clip(u + D*lap(u) + rho*u*(1-u), 0, 1)
       = clip(D*E + (1 + rho - 4D)*u - rho*u^2, 0, 1),   E = 4-neighbor sum

Distribution: 32 samples -> 8 cores x 4 samples, banded by sorted t so the
uniform SPMD program's per-step active width w_k = max_c n_{c,k} is minimal.
Since widths are non-increasing, the schedule is four width-phases (w=4,3,2,1)
each implemented as a Tile For_i dynamic loop -> the program stays small
(program load cost here is ~25-60us per instruction, so unrolling 30 steps is
far more expensive than loop back-edges).

Per-core SBUF layout: [128, 2064] f32 = 4 samples x 2 chunks x (1+256+1) cols
(guard columns keep the horizontal stencil one strided op). Chunk0 holds field
rows 0..127 partition-REVERSED (partition p = row 127-p), chunk1 rows 128..255
naturally, so both cross-chunk halo terms live at partition 0 (engine SBUF
reads must start at a 32-aligned partition base; only outputs may be offset).
In-chunk vertical neighbor sums run on TensorE as S_v @ u (S_v tridiagonal 0/1
fp32, stationary) into PSUM; halos are one-partition DVE adds accumulated into
PSUM at base 0.

Per-sample early finish is handled with an on-device step counter K and a
death-step field Dt: mask8 = (Dt > K) (int8), consumed by copy_predicated via
a 0-stride broadcast AP, freezing finished samples bit-exactly."""

import numpy as np

import concourse.bass as bass
import concourse.mybir as mybir
import concourse.tile as tile
from concourse.bass_utils import run_bass_kernel_spmd

N_CORES = 8
B, H, W = 32, 256, 256
NPOS = 4                    # samples per core
BLK = W + 2                 # 258: guard + 256 data + guard
SCOLS = 2 * BLK             # 516 cols per sample
COLS = NPOS * SCOLS         # 2064
DATA = 2 * W                # 512 data cols per sample (PSUM layout)
F32 = mybir.dt.float32

MAX_WAITS = 1               # walrus CTRL codegen: at most 1 sync wait per inst


def _split_excess_waits(nc):
    """Spill extra sem waits onto Drain instructions inserted before the
    offending instruction (walrus supports one sync wait per CTRL inst)."""
    for bb in nc.main_func.blocks:
        insts = list(bb.instructions)
        new_list = []
        changed = False
        for ins in insts:
            si = ins.sync_info
            if si is not None and si.on_wait and len(si.on_wait) > MAX_WAITS:
                waits = list(si.on_wait)
                extras, keep = waits[:-MAX_WAITS], waits[-MAX_WAITS:]
                while extras:
                    chunk, extras = extras[:MAX_WAITS], extras[MAX_WAITS:]
                    new_list.append(
                        mybir.InstDrain(
                            name=f"I-waitsplit-{nc.next_id()}",
                            engine=ins.engine,
                            sync_info=mybir.SyncInfo(on_wait=chunk, on_update=[]),
                            bass_is_fusable=False,
                        )
                    )
                si.on_wait = keep
                ins.sync_info = si
                changed = True
            new_list.append(ins)
        if changed:
            bb.instructions = new_list
    return nc


def _build_program(phases, outer_repeat=1, ablate_mm=False, staggered=False, unroll=4, pool_offload=True):
    """phases: list of (width, ntrips). outer_repeat wraps the whole schedule
    in a timing loop (iterations beyond the first are identity: every sample's
    mask is already 0), used by the HW-time estimator."""
    AL = mybir.AluOpType
    nc = bass.Bass()

    x_in = nc.declare_dram_parameter("x", [128, COLS], F32, isOutput=False)
    sv_in = nc.declare_dram_parameter("sv", [128, 128], F32, isOutput=False)
    d_in = nc.declare_dram_parameter("d", [128, COLS], F32, isOutput=False)
    c_in = nc.declare_dram_parameter("c", [128, COLS], F32, isOutput=False)
    r_in = nc.declare_dram_parameter("r", [128, COLS], F32, isOutput=False)
    dt_in = nc.declare_dram_parameter("dt", [128, 2 * NPOS], F32, isOutput=False)
    y_out = nc.declare_dram_parameter("y", [128, COLS], F32, isOutput=True)

    with tile.TileContext(nc) as tc:
        with (
            tc.tile_pool(name="pool", bufs=1) as pool,
            tc.tile_pool(name="psum", bufs=1, space="PSUM") as psum,
        ):
            X = pool.tile([128, COLS], F32)
            SV = pool.tile([128, 128], F32)
            DD = pool.tile([128, COLS], F32)
            CC = pool.tile([128, COLS], F32)
            RR = pool.tile([128, COLS], F32)
            DT = pool.tile([128, 2 * NPOS], F32)
            KC = pool.tile([128, 1], F32)
            M8 = pool.tile([128, 2 * NPOS], mybir.dt.int8)
            HS = pool.tile([128, COLS], F32)
            T2 = pool.tile([128, COLS], F32)
            SQ = pool.tile([128, COLS], F32)
            PS = psum.tile([128, NPOS * DATA], F32)

            nc.sync.dma_start(X[:], x_in[:])
            nc.sync.dma_start(SV[:], sv_in[:])
            nc.sync.dma_start(DT[:], dt_in[:])
            nc.sync.dma_start(DD[:], d_in[:])
            nc.sync.dma_start(CC[:], c_in[:])
            nc.sync.dma_start(RR[:], r_in[:])

            nc.vector.memset(KC[:], 0.0)
            # HS guard columns stay 0 forever: every op below either writes
            # data columns only or multiplies a zero-guard field in.
            nc.vector.memset(HS[:], 0.0)

            X4 = X[:].rearrange("p (s q c) -> p s q c", q=2, c=BLK)
            X3 = X[:].rearrange("p (b c) -> p b c", c=BLK)
            H4 = HS[:].rearrange("p (s q c) -> p s q c", q=2, c=BLK)
            H3 = HS[:].rearrange("p (b c) -> p b c", c=BLK)
            P4 = PS[:].rearrange("p (s q c) -> p s q c", q=2, c=W)

            def step_body(w, masked=True):
                nb = 2 * w
                cw = SCOLS * w

                # ScalarE: u^2 (contiguous keeps SQ guards = 0)
                nc.scalar.square(SQ[:, :cw], X[:, :cw])
                # mask = (Dt > K) on DVE (Pool rejects f32->int8 compares);
                # t2 = C*u on GpSimd (parallel with DVE chain)
                if masked:
                    nc.vector.tensor_tensor(
                        M8[:, :nb], DT[:, :nb], KC[:].broadcast_to([128, nb]), AL.is_gt
                    )
                eng2 = nc.gpsimd if pool_offload else nc.vector
                eng2.tensor_mul(T2[:, :cw], CC[:, :cw], X[:, :cw])
                # t4 = rho * u^2 (after ScalarE sq)
                eng2.tensor_mul(SQ[:, :cw], RR[:, :cw], SQ[:, :cw])

                # DVE: horizontal neighbor sum via guard bands (emitted
                # before the matmuls so the DVE never idles waiting on PSUM)
                nc.vector.tensor_add(
                    H3[:, :nb, 1 : 1 + W], X3[:, :nb, 0:W], X3[:, :nb, 2 : 2 + W]
                )
                # TensorE: in-chunk vertical neighbor sums -> PSUM
                for j in ([] if ablate_mm else range(w)):
                    nc.tensor.matmul(
                        PS[:, DATA * j : DATA * (j + 1)],
                        SV[:],
                        X4[:, j, :, 1 : 1 + W],
                        start=True,
                        stop=True,
                    )
                # DVE: both cross-chunk halos in one op — partition 0 of each
                # chunk adds the *other* chunk's partition-0 row (block-swap
                # via negative free-dim stride).
                xswap = bass.AP(
                    tensor=X4.tensor,
                    offset=X4.offset + BLK + 1,
                    ap=[[X4.ap[0][0], 1], [SCOLS, w], [-BLK, 2], [1, W]],
                )
                nc.vector.tensor_add(P4[0:1, :w, :, :], P4[0:1, :w, :, :], xswap)
                # E = hs + vs
                nc.vector.tensor_add(
                    H4[:, :w, :, 1 : 1 + W], H4[:, :w, :, 1 : 1 + W], P4[:, :w, :, :]
                )
                # g = C*u - rho*u^2 on GpSimd; DVE: r = D*E + g
                eng2.tensor_sub(T2[:, :cw], T2[:, :cw], SQ[:, :cw])
                nc.vector.tensor_mul(HS[:, :cw], DD[:, :cw], HS[:, :cw])
                nc.vector.tensor_add(HS[:, :cw], HS[:, :cw], T2[:, :cw])

                if masked:
                    # clip into scratch, then predicated-write active samples
                    nc.vector.tensor_scalar(
                        HS[:, :cw], HS[:, :cw], 0.0, 1.0, AL.max, AL.min
                    )
                    mb = (
                        M8[:, :nb]
                        .rearrange("p (s q) -> p s q", q=2)
                        .broadcast_to([128, w, 2, W])
                    )
                    nc.vector.copy_predicated(
                        X4[:, :w, :, 1 : 1 + W], mb, H4[:, :w, :, 1 : 1 + W]
                    )
                else:
                    # every in-width sample is still alive: clip straight to u
                    nc.vector.tensor_scalar(
                        X4[:, :w, :, 1 : 1 + W],
                        H4[:, :w, :, 1 : 1 + W],
                        0.0,
                        1.0,
                        AL.max,
                        AL.min,
                    )

                # ScalarE: K += 1 (after mask use)
                nc.scalar.add(KC[:], KC[:], 1.0)

            def schedule_body():
                for w, n_free, n_masked in phases:
                    for ntrips, masked in ((n_free, False), (n_masked, True)):
                        if ntrips <= 0:
                            continue
                        reps, rem = divmod(ntrips, unroll)
                        if reps == 1:
                            for _ in range(unroll):
                                step_body(w, masked)
                        elif reps > 1:
                            with tc.For_i(0, reps, 1, staggered_reset=staggered):
                                for _ in range(unroll):
                                    step_body(w, masked)
                        for _ in range(rem):
                            step_body(w, masked)

            if outer_repeat > 1:
                with tc.For_i(0, outer_repeat, 1, staggered_reset=staggered):
                    schedule_body()
            else:
                schedule_body()

            nc.sync.dma_start(y_out[:], X[:])

    return _split_excess_waits(nc)


_program_cache = {}


def _get_program(phases, outer_repeat=1, ablate_mm=False, staggered=False, unroll=4, pool_offload=True):
    key = (tuple(phases), outer_repeat, ablate_mm, staggered, unroll, pool_offload)
    prog = _program_cache.get(key)
    if prog is None:
        prog = _build_program(phases, outer_repeat, ablate_mm, staggered, unroll, pool_offload)
        _program_cache[key] = prog
    return prog


def _schedule(t):
    """Banded assignment + width phases. Returns (phases, samples) where
    samples[c][j] = global sample index for core c, position j."""
    order = np.argsort(-t, kind="stable")
    band_max = [int(t[order[8 * j]]) for j in range(NPOS)]  # b0>=b1>=b2>=b3
    b0, b1, b2, b3 = band_max
    bounds = [(4, 0, b3), (3, b3, b2), (2, b2, b1), (1, b1, b0)]
    phases = []
    for w, start, end in bounds:
        n = end - start
        if n <= 0:
            phases.append((w, 0, 0))
            continue
        # samples inside the width = everything in bands < w; no mask is
        # needed while all of them are still alive (k < their min t)
        min_t = int(min(t[order[i]] for i in range(8 * w)))
        n_free = min(max(min_t - start, 0), n)
        phases.append((w, n_free, n - n_free))
    samples = [[int(order[8 * j + c]) for j in range(NPOS)] for c in range(N_CORES)]
    return phases, samples


def _pack_field(dst, field, j):
    """Place one sample's [256, 256] field into dst [128, COLS] at position j
    (chunk0 partition-reversed, chunk1 natural); guards stay untouched."""
    off = SCOLS * j
    dst[:, off + 1 : off + 1 + W] = field[127::-1, :]
    dst[:, off + BLK + 1 : off + BLK + 1 + W] = field[128:, :]


def _make_in_maps(u, params, t, samples):
    sv = np.zeros((128, 128), np.float32)
    for i in range(127):
        sv[i, i + 1] = 1.0
        sv[i + 1, i] = 1.0
    D = params[:, 0]
    rho = params[:, 1]
    C = (1.0 + rho - 4.0 * D).astype(np.float32)
    in_maps = []
    for c in range(N_CORES):
        X = np.zeros((128, COLS), np.float32)
        Dp = np.zeros((128, COLS), np.float32)
        Cp = np.zeros((128, COLS), np.float32)
        Rp = np.zeros((128, COLS), np.float32)
        dt = np.zeros((128, 2 * NPOS), np.float32)
        for j in range(NPOS):
            s = samples[c][j]
            _pack_field(X, u[s, 0], j)
            _pack_field(Dp, D[s], j)
            _pack_field(Cp, C[s], j)
            _pack_field(Rp, rho[s], j)
            dt[:, 2 * j] = float(t[s])
            dt[:, 2 * j + 1] = float(t[s])
        in_maps.append({"x": X, "sv": sv, "d": Dp, "c": Cp, "r": Rp, "dt": dt})
    return in_maps


def kernel(u, params, t):
    u = np.ascontiguousarray(np.asarray(u, dtype=np.float32))
    params = np.ascontiguousarray(np.asarray(params, dtype=np.float32))
    t = np.asarray(t, dtype=np.int32)

    T = int(t.max()) if t.size else 0
    if T <= 0:
        return u.copy()

    phases, samples = _schedule(t)
    nc = _get_program(phases)
    in_maps = _make_in_maps(u, params, t, samples)
    res = run_bass_kernel_spmd(nc, in_maps, list(range(N_CORES)))

    out = np.empty((B, 1, H, W), np.float32)
    for c in range(N_CORES):
        Y = res.results[c]["y"]
        for j in range(NPOS):
            s = samples[c][j]
            off = SCOLS * j
            out[s, 0, :128, :] = Y[:, off + 1 : off + 1 + W][::-1, :]
            out[s, 0, 128:, :] = Y[:, off + BLK + 1 : off + BLK + 1 + W]
    return out
